# revision 19
# baseline (speedup 1.0000x reference)
"""DecoderRNN Trainium2 kernel (8-core data-parallel).

Shards batch 1024 -> 128 per NeuronCore. Weights replicated. The LSTM scan
runs locally per shard; the encoder-outputs "num" head is interleaved into
tensor-engine gaps of the scan (two of its four m-tiles per step).

Key layout/precision choices (per core, Bs=128):
 - Matmuls use float32r (full PE rate for moving free-dim >= 256, ~1.8e-4
   operand rounding); PE transposes stay plain fp32 (exact).
 - LSTM state kept natural [Bs, H]; transposed copies hT/xT [H, Bs] are
   rebuilt each step via PE transposes because both gate matmuls need K=H on
   partitions.
 - All transcendentals are Tanh so the ACT engine never reloads its LUT
   (LoadActFuncSet costs ~1.3us per switch): sigmoid(x) = (tanh(x/2)+1)/2.
   To absorb the resulting factors of two the carried state is H = 2h,
   D = 2c, with W_ih/W_hh/W_out pre-scaled by 1/2 and W_lat2hid{,2} (and
   their biases) by 2 on the host; outputs are halved once at the end.
 - leaky-relu / relu run on the vector engine (tensor_scalar + stt), again
   avoiding ACT LUT switches.
 - start-token contribution and any nonzero biases enter via K=1 matmuls
   with a ones row (broadcast along partitions).
 - num head: encoder outputs are pre-transposed on host to [L, rows] so the
   moving operand streams contiguously; chunks of 512 rows, DMA prefetched
   two chunks ahead.
"""

import numpy as np

import concourse.bass as bass
import concourse.tile as tile
from concourse import bacc, mybir
from concourse.bass_utils import run_bass_kernel_spmd
from concourse.masks import make_identity

FP32 = mybir.dt.float32
F32R = mybir.dt.float32r

NCORES = 8
B, T_ENC, L, H, O = 1024, 200, 256, 512, 128
BS = B // NCORES          # 128 batch rows per core
KH = H // 128             # 4 k-tiles over H
KL = L // 128             # 2 k-tiles over L
R = BS * T_ENC            # 25600 num-head rows per core
CHUNK = 512               # num-head rows per chunk
NCH = R // CHUNK          # 50 chunks
OPAD = 256                # y-projection moving width (>=256 keeps f32r fast)

# gate chunk order g, i, f, o: the c-update chain completes while o streams
GATE_ORDER = (2, 0, 1, 3)

PROFILE = False
LAST_RESULTS = None
LAST_IN_MAPS = None


def build_program(seq_len=100, use_gate_bias=False, use_out_bias=False):
    nc = bacc.Bacc("TRN2", target_bir_lowering=False, debug=False)

    d = {}
    def din(name, shape, dt=F32R):
        d[name] = nc.dram_tensor(name, shape, dt, kind="ExternalInput").ap()
    def dout(name, shape, dt=FP32):
        d[name] = nc.dram_tensor(name, shape, dt, kind="ExternalOutput").ap()

    din("wih_t", [KH, 128, 4 * H])        # (W_ih/2).T  k-tiled
    din("whh_t", [KH, 128, 4 * H])
    din("wout_t", [KH, 128, OPAD])        # (W_out/2).T zero-padded to OPAD
    din("wseq_t", [KL, 128, H])
    din("wl2h_t", [KL, 128, H])           # (2 W_lat2hid).T
    din("wl2h2_t", [KL, 128, H])
    din("w2_t", [KH, 128, 1])
    din("eh_t", [KL, 128, BS])            # encoder_hidden shard, transposed
    din("x_t", [KL, 128, R])              # encoder_outputs shard, transposed
    din("g0", [1, 4 * H])                 # relu(x0) @ W_ih.T + b_ih + b_hh
    din("bias_g", [1, 4 * H])             # b_ih + b_hh (emitted if nonzero)
    din("bias_out", [1, OPAD])
    din("bias_h0", [1, H])                # 2*b_lat2hid (K=1 row into h0 psum)
    din("bias_c0", [1, H])
    din("b_seq", [128, KH], FP32)         # b_seq laid out [128, 4] per h-tile
    din("b_seq2", [1, 1], FP32)

    dout("dec_out", [BS, seq_len, O])
    dout("h_out", [BS, H])
    dout("c_out", [BS, H])
    dout("num_out", [1, R])

    with tile.TileContext(nc) as tc:
        _emit(tc, nc, d, seq_len, use_gate_bias, use_out_bias)
    nc.compile()
    return nc


def _emit(tc, nc, d, seq_len, use_gate_bias, use_out_bias):
    import contextlib
    ctx = contextlib.ExitStack()
    with ctx:
        wpool = ctx.enter_context(tc.tile_pool(name="weights", bufs=1))
        spool = ctx.enter_context(tc.tile_pool(name="state", bufs=1))
        apool = ctx.enter_context(tc.tile_pool(name="acts", bufs=2))
        xpool = ctx.enter_context(tc.tile_pool(name="xenc", bufs=3))
        ypool = ctx.enter_context(tc.tile_pool(name="ystage", bufs=2))
        gps = ctx.enter_context(tc.tile_pool(name="gatesps", bufs=3, space="PSUM"))
        scr = ctx.enter_context(tc.tile_pool(name="scratch", bufs=5, space="PSUM"))

        AF = mybir.ActivationFunctionType
        ALU = mybir.AluOpType

        # ---- resident weights ----
        wih = wpool.tile([128, KH, 4 * H], F32R, tag="wih")
        whh = wpool.tile([128, KH, 4 * H], F32R, tag="whh")
        wout = wpool.tile([128, KH, OPAD], F32R, tag="wout")
        wseq = wpool.tile([128, KL, H], F32R, tag="wseq")
        wl2h = wpool.tile([128, KL, H], F32R, tag="wl2h")
        wl2h2 = wpool.tile([128, KL, H], F32R, tag="wl2h2")
        w2 = wpool.tile([128, KH, 1], F32R, tag="w2")
        eht = wpool.tile([128, KL, BS], F32R, tag="eht")
        g0 = wpool.tile([1, 4 * H], F32R, tag="g0")
        bseq = wpool.tile([128, KH], FP32, tag="bseq")
        bseq2 = wpool.tile([1, 1], FP32, tag="bseq2")
        for k in range(KL):
            nc.sync.dma_start(eht[:, k, :], d["eh_t"][k])
            nc.sync.dma_start(wl2h[:, k, :], d["wl2h_t"][k])
            nc.sync.dma_start(wl2h2[:, k, :], d["wl2h2_t"][k])
            nc.sync.dma_start(wseq[:, k, :], d["wseq_t"][k])
        for k in range(KH):
            nc.sync.dma_start(whh[:, k, :], d["whh_t"][k])
            nc.sync.dma_start(wih[:, k, :], d["wih_t"][k])
            nc.sync.dma_start(wout[:, k, :], d["wout_t"][k])
            nc.sync.dma_start(w2[:, k, :], d["w2_t"][k])
        nc.sync.dma_start(g0[:], d["g0"][:])
        nc.sync.dma_start(bseq[:], d["b_seq"][:])
        nc.sync.dma_start(bseq2[:], d["b_seq2"][:])

        bias_g = bias_out = None
        if use_gate_bias:
            bias_g = wpool.tile([1, 4 * H], F32R, tag="bias_g")
            nc.sync.dma_start(bias_g[:], d["bias_g"][:])
        if use_out_bias:
            bias_out = wpool.tile([1, OPAD], F32R, tag="bias_out")
            nc.sync.dma_start(bias_out[:], d["bias_out"][:])
        bias_h0 = wpool.tile([1, H], F32R, tag="bias_h0")
        bias_c0 = wpool.tile([1, H], F32R, tag="bias_c0")
        nc.sync.dma_start(bias_h0[:], d["bias_h0"][:])
        nc.sync.dma_start(bias_c0[:], d["bias_c0"][:])

        ident = wpool.tile([128, 128], FP32, tag="ident")
        make_identity(nc, ident[:])
        ones_f = wpool.tile([1, 128], FP32, tag="ones_f")
        nc.vector.memset(ones_f[:], 1.0)
        ones = wpool.tile([1, 128], F32R, tag="ones")
        nc.vector.tensor_copy(ones[:], ones_f[:])

        # ---- persistent state (H = 2h, D = 2c) ----
        hst = spool.tile([128, H], FP32, tag="hst")
        dst_c = spool.tile([128, H], FP32, tag="dst_c")
        hT = spool.tile([128, KH, 128], F32R, tag="hT")
        xT = spool.tile([128, KH, 128], F32R, tag="xT")

        def dve_lrelu(out_ap, ps_ap, bias=None):
            # out = lrelu(ps + bias); two DVE ops, no ACT LUT switch
            t = apool.tile([ps_ap.shape[0], ps_ap.shape[-1]], FP32, tag="lrtmp")
            if bias is None:
                nc.vector.tensor_copy(t[:], ps_ap)
            else:
                nc.vector.tensor_scalar(t[:], ps_ap, bias, None, ALU.add)
            nc.vector.scalar_tensor_tensor(out_ap, t[:], 0.01, t[:],
                                           ALU.mult, ALU.max)

        # ---- h0 / c0 (state enters pre-doubled via the 2x-scaled weights) ----
        for dst, w, brow in ((hst, wl2h, bias_h0), (dst_c, wl2h2, bias_c0)):
            ps = scr.tile([128, H], FP32, tag="scr")
            for k in range(KL):
                nc.tensor.matmul(ps[:], eht[:, k, :], w[:, k, :],
                                 start=(k == 0), stop=False)
            nc.tensor.matmul(ps[:], ones[:], brow[:], start=False, stop=True)
            dve_lrelu(dst[:], ps[:])

        for k in range(KH):
            tp = scr.tile([128, 128], FP32, tag="scr")
            nc.tensor.transpose(tp[:], hst[:, k * 128:(k + 1) * 128], ident[:])
            nc.scalar.activation(hT[:, k, :], tp[:], AF.Copy)

        # ---- num head machinery (software-pipelined into the scan) ----
        xe_tiles = {}
        act_tiles = {}
        nhalf_tiles = {}

        def prefetch_chunk(cidx):
            if cidx >= NCH:
                return
            xe = xpool.tile([128, KL, CHUNK], F32R, tag="xe")
            for k in range(KL):
                nc.sync.dma_start(
                    xe[:, k, :], d["x_t"][k][:, cidx * CHUNK:(cidx + 1) * CHUNK])
            xe_tiles[cidx] = xe

        def num_hs(cidx, m):
            # first-layer matmuls for one m-tile (PE); psum kept for the lrelu
            xe = xe_tiles[cidx]
            ps = scr.tile([128, CHUNK], FP32, tag="scr", name="numps")
            for k in range(KL):
                nc.tensor.matmul(ps[:], wseq[:, k, m * 128:(m + 1) * 128],
                                 xe[:, k, :], start=(k == 0), stop=(k == KL - 1))
            if m == KH - 1:
                del xe_tiles[cidx]
            return ps

        def num_lrelu(cidx, m, ps):
            a = apool.tile([128, CHUNK], F32R, tag="numact", bufs=4)
            dve_lrelu(a[:], ps[:], bias=bseq[:, m:m + 1])
            act_tiles[(cidx, m)] = a

        def num_tail_mm(pair):
            # second-layer accumulation (PE), one step behind the lrelus;
            # each half-pair finishes in its own psum slot (copied/reduced to
            # SBUF the same step, so nothing pins psum across steps)
            (cidx, m0), (_, m1) = pair
            nt = scr.tile([1, CHUNK], FP32, tag="scr", name="ntile")
            for j, m in enumerate((m0, m1)):
                a = act_tiles.pop((cidx, m))
                nc.tensor.matmul(nt[:], w2[:, m, :], a[:],
                                 start=(j == 0), stop=(j == 1))
            return cidx, m1, nt

        def num_tail_dve(cidx, m1, nt):
            if m1 == 1:
                nh = apool.tile([1, CHUNK], FP32, tag="nhalf", bufs=2)
                nc.vector.tensor_copy(nh[:], nt[:])
                nhalf_tiles[cidx] = nh
            else:
                nh = nhalf_tiles.pop(cidx)
                t2 = apool.tile([1, CHUNK], FP32, tag="ntmp", bufs=2)
                nc.vector.scalar_tensor_tensor(t2[:], nh[:], bseq2[0:1, 0:1],
                                               nt[:], ALU.add, ALU.add)
                no = ypool.tile([1, CHUNK], FP32, tag="numout")
                nc.vector.tensor_scalar(no[:], t2[:], 0.0, None, ALU.max)
                nc.sync.dma_start(
                    d["num_out"][:, cidx * CHUNK:(cidx + 1) * CHUNK], no[:])

        prefetch_chunk(0)
        prefetch_chunk(1)
        pend = []

        # ---- the scan ----
        for t in range(seq_len):
            # gates = 2relu(h) @ (W_ih/2).T + 2h @ (W_hh/2).T (+ bias)
            # each chunk accumulates in its own rotating psum tile so the
            # tanh that consumes it has a precise, early dependency
            gch = {}
            for n in GATE_ORDER:
                gch[n] = gps.tile([128, 512], FP32, tag="gch", name="gch")
                gsl = gch[n][:]
                nsl = slice(n * 512, (n + 1) * 512)
                for k in range(KH):
                    nc.tensor.matmul(gsl, hT[:, k, :], whh[:, k, nsl],
                                     start=(k == 0), stop=False)
                if t == 0:
                    nc.tensor.matmul(gsl, ones[:], g0[:, nsl],
                                     start=False, stop=True)
                else:
                    if use_gate_bias:
                        nc.tensor.matmul(gsl, ones[:], bias_g[:, nsl],
                                         start=False, stop=False)
                    for k in range(KH):
                        nc.tensor.matmul(gsl, xT[:, k, :],
                                         wih[:, k, nsl], start=False,
                                         stop=(k == KH - 1))

            # first-layer num matmuls + deferred second layer fill the
            # gate->state dependency window on the PE
            cidx, half = divmod(t, 2)
            work = [(cidx, 2 * half), (cidx, 2 * half + 1)] if cidx < NCH else []
            hs_ps = [num_hs(c, m) for c, m in work]
            if work and half == 0:
                prefetch_chunk(cidx + 2)
            tail = num_tail_mm(pend) if pend else None

            # tanh-only nonlinearities: sig(x) = (tanh(x/2)+1)/2
            th_g = apool.tile([128, 512], FP32, tag="th_g")
            th_i = apool.tile([128, 512], FP32, tag="th_i")
            th_f = apool.tile([128, 512], FP32, tag="th_f")
            nc.scalar.activation(th_g[:], gch[2][:], AF.Tanh)
            nc.scalar.activation(th_i[:], gch[0][:], AF.Tanh, scale=0.5)
            nc.scalar.activation(th_f[:], gch[1][:], AF.Tanh, scale=0.5)
            th_o = apool.tile([128, 512], FP32, tag="th_o")

            # D = 2c update: D' = 0.5(th_f+1)D + (th_i+1)th_g
            tmp = apool.tile([128, 512], FP32, tag="tmp")
            nc.vector.scalar_tensor_tensor(tmp[:], th_i[:], 1.0, th_g[:],
                                           ALU.add, ALU.mult)
            c2 = apool.tile([128, 512], FP32, tag="c2")
            nc.vector.scalar_tensor_tensor(c2[:], th_f[:], 1.0, dst_c[:],
                                           ALU.add, ALU.mult)
            nc.vector.scalar_tensor_tensor(dst_c[:], c2[:], 0.5, tmp[:],
                                           ALU.mult, ALU.add)
            th_c = apool.tile([128, 512], FP32, tag="th_c")
            nc.scalar.activation(th_c[:, 0:256], dst_c[:, 0:256], AF.Tanh, scale=0.5)
            nc.scalar.activation(th_c[:, 256:512], dst_c[:, 256:512], AF.Tanh, scale=0.5)
            nc.scalar.activation(th_o[:, 0:256], gch[3][:, 0:256], AF.Tanh, scale=0.5)
            nc.scalar.activation(th_o[:, 256:512], gch[3][:, 256:512], AF.Tanh, scale=0.5)
            # H = 2h = (th_o+1) tanh(c), in k-quarters so transposes start early
            for k in range(KH):
                ksl = slice(k * 128, (k + 1) * 128)
                nc.vector.scalar_tensor_tensor(hst[:, ksl], th_o[:, ksl], 1.0,
                                               th_c[:, ksl], ALU.add, ALU.mult)
            if tail is not None:
                num_tail_dve(*tail)

            # rebuild transposed state; copies alternate ACT/DVE so the y
            # matmuls are not serialized behind one engine
            tps = []
            for k in range(KH):
                tp = scr.tile([128, 128], FP32, tag="scr", name="tp")
                nc.tensor.transpose(tp[:], hst[:, k * 128:(k + 1) * 128], ident[:])
                if k % 2 == 0:
                    nc.scalar.activation(hT[:, k, :], tp[:], AF.Copy)
                else:
                    nc.vector.tensor_copy(hT[:, k, :], tp[:])
                tps.append(tp)

            # y = H @ (W_out/2).T  (moving side padded to OPAD for f32r rate)
            yps = scr.tile([128, OPAD], FP32, tag="scr", name="yps")
            for k in range(KH):
                nc.tensor.matmul(yps[:], hT[:, k, :], wout[:, k, :],
                                 start=(k == 0), stop=(k == KH - 1 and not use_out_bias))
            if use_out_bias:
                nc.tensor.matmul(yps[:], ones[:], bias_out[:], start=False, stop=True)

            if t + 1 < seq_len:
                for k in range(KH):
                    nc.vector.tensor_relu(xT[:, k, :], tps[k][:])
            y = ypool.tile([128, O], FP32, tag="y")
            nc.scalar.activation(y[:], yps[:, :O], AF.Copy)
            nc.sync.dma_start(d["dec_out"][:, t, :], y[:])
            for (c, m), ps in zip(work, hs_ps):
                num_lrelu(c, m, ps)
            pend = work

        # drain the deferred num tail (and any chunks past the scan)
        if pend:
            num_tail_dve(*num_tail_mm(pend))
        cidx = seq_len // 2
        while cidx < NCH:
            prefetch_chunk(cidx)
            for half in range(2):
                pair = [(cidx, 2 * half), (cidx, 2 * half + 1)]
                for c, m in pair:
                    num_lrelu(c, m, num_hs(c, m))
                num_tail_dve(*num_tail_mm(pair))
            cidx += 1

        # outputs carry the true h = H/2, c = D/2
        ho = ypool.tile([128, H], FP32, tag="houtst")
        nc.vector.tensor_scalar(ho[:], hst[:], 0.5, None, mybir.AluOpType.mult)
        nc.sync.dma_start(d["h_out"][:], ho[:])
        co = ypool.tile([128, H], FP32, tag="coutst")
        nc.vector.tensor_scalar(co[:], dst_c[:], 0.5, None, mybir.AluOpType.mult)
        nc.sync.dma_start(d["c_out"][:], co[:])


# ---------------------------------------------------------------------------
# host side
# ---------------------------------------------------------------------------

_CACHE = {}


def _prep_host(inputs):
    f32 = lambda a: np.ascontiguousarray(np.asarray(a), dtype=np.float32)
    W_ih, W_hh = f32(inputs["W_ih"]), f32(inputs["W_hh"])
    b_ih, b_hh = f32(inputs["b_ih"]), f32(inputs["b_hh"])
    W_out, b_out = f32(inputs["W_out"]), f32(inputs["b_out"])
    W_emb, b_emb = f32(inputs["W_emb"]), f32(inputs["b_emb"])
    W_seq, b_seq = f32(inputs["W_seq"]), f32(inputs["b_seq"])
    W_seq2, b_seq2 = f32(inputs["W_seq2"]), f32(inputs["b_seq2"])
    W_l2h, b_l2h = f32(inputs["W_lat2hid"]), f32(inputs["b_lat2hid"])
    W_l2h2, b_l2h2 = f32(inputs["W_lat2hid2"]), f32(inputs["b_lat2hid2"])
    start = f32(inputs["start_token"])

    x0 = start @ W_emb.T + b_emb                      # [1, H]
    bias_g = (b_ih + b_hh)[None, :]                   # [1, 4H]
    g0 = np.maximum(x0, 0.0) @ W_ih.T + bias_g        # [1, 4H] exact

    wout_pad = np.zeros((H, OPAD), np.float32)
    wout_pad[:, :O] = 0.5 * W_out.T
    bout_pad = np.zeros((1, OPAD), np.float32)
    bout_pad[0, :O] = b_out

    shared = {
        "wih_t": np.ascontiguousarray(0.5 * W_ih.T).reshape(KH, 128, 4 * H),
        "whh_t": np.ascontiguousarray(0.5 * W_hh.T).reshape(KH, 128, 4 * H),
        "wout_t": np.ascontiguousarray(wout_pad).reshape(KH, 128, OPAD),
        "wseq_t": np.ascontiguousarray(W_seq.T).reshape(KL, 128, H),
        "wl2h_t": np.ascontiguousarray(2.0 * W_l2h.T).reshape(KL, 128, H),
        "wl2h2_t": np.ascontiguousarray(2.0 * W_l2h2.T).reshape(KL, 128, H),
        "w2_t": np.ascontiguousarray(W_seq2.T).reshape(KH, 128, 1),
        "g0": np.ascontiguousarray(g0),
        "bias_g": np.ascontiguousarray(bias_g),
        "bias_out": bout_pad,
        "bias_h0": np.ascontiguousarray(2.0 * b_l2h[None, :]),
        "bias_c0": np.ascontiguousarray(2.0 * b_l2h2[None, :]),
        "b_seq": np.ascontiguousarray(b_seq.reshape(KH, 128).T),
        "b_seq2": np.ascontiguousarray(b_seq2.reshape(1, 1)),
    }
    flags = dict(
        use_gate_bias=bool(np.any(bias_g != 0)),
        use_out_bias=bool(np.any(b_out != 0)),
    )
    return shared, flags


def kernel(**inputs):
    enc = np.ascontiguousarray(np.asarray(inputs["encoder_outputs"]), dtype=np.float32)
    eh = np.ascontiguousarray(np.asarray(inputs["encoder_hidden"]), dtype=np.float32)
    seq_len = int(inputs["seq_len"])

    shared, flags = _prep_host(inputs)

    key = (seq_len, tuple(sorted(flags.items())))
    if key not in _CACHE:
        _CACHE[key] = build_program(seq_len=seq_len, **flags)
    nc = _CACHE[key]

    in_maps = []
    for cid in range(NCORES):
        sl = slice(cid * BS, (cid + 1) * BS)
        m = dict(shared)
        m["eh_t"] = np.ascontiguousarray(eh[sl].T).reshape(KL, 128, BS)
        m["x_t"] = np.ascontiguousarray(
            enc[sl].reshape(R, L).T).reshape(KL, 128, R)
        in_maps.append(m)

    global LAST_RESULTS, LAST_IN_MAPS
    LAST_IN_MAPS = in_maps
    res = run_bass_kernel_spmd(nc, in_maps, core_ids=list(range(NCORES)),
                               trace=PROFILE)
    LAST_RESULTS = res

    dec = np.concatenate([res.results[i]["dec_out"] for i in range(NCORES)], axis=0)
    hT = np.concatenate([res.results[i]["h_out"] for i in range(NCORES)], axis=0)
    cT = np.concatenate([res.results[i]["c_out"] for i in range(NCORES)], axis=0)
    num = np.concatenate(
        [res.results[i]["num_out"].reshape(BS, T_ENC) for i in range(NCORES)],
        axis=0)[..., None]
    return dec, hT[None], cT[None], num


# revision 29
# speedup vs baseline: 17.9089x; 17.9089x over previous
"""DecoderRNN Trainium2 kernel (8-core data-parallel).

Shards batch 1024 -> 128 per NeuronCore. Weights replicated. The LSTM scan
runs locally per shard; the encoder-outputs "num" head is interleaved into
tensor-engine gaps of the scan (two of its four m-tiles per step).

Key layout/precision choices (per core, Bs=128):
 - Matmuls and PE transposes use float32r (full PE rate for moving
   free-dim >= 256; ~1.8e-4 operand rounding inside the PE).
 - LSTM state kept natural [Bs, H]; transposed copies hT/xT [H, Bs] are
   rebuilt each step via PE transposes because both gate matmuls need K=H on
   partitions.
 - All transcendentals are Tanh so the ACT engine never reloads its LUT
   (LoadActFuncSet costs ~1.3us per switch): sigmoid(x) = (tanh(x/2)+1)/2.
   To absorb the resulting factors of two the carried state is H = 2h,
   D = 2c, with W_ih/W_hh/W_out pre-scaled by 1/2 and W_lat2hid{,2} (and
   their biases) by 2 on the host; outputs are halved once at the end.
 - leaky-relu / relu run on the vector engine (tensor_scalar + stt), again
   avoiding ACT LUT switches.
 - start-token contribution and any nonzero biases enter via K=1 matmuls
   with a ones row (broadcast along partitions).
 - num head: encoder outputs are pre-transposed on host to [L, rows] so the
   moving operand streams contiguously; chunks of 512 rows, DMA prefetched
   two chunks ahead.
"""

import numpy as np

import concourse.bass as bass
import concourse.tile as tile
from concourse import bacc, mybir
from concourse.bass_utils import run_bass_kernel_spmd
from concourse.masks import make_identity

FP32 = mybir.dt.float32
F32R = mybir.dt.float32r

NCORES = 8
B, T_ENC, L, H, O = 1024, 200, 256, 512, 128
BS = B // NCORES          # 128 batch rows per core
KH = H // 128             # 4 k-tiles over H
KL = L // 128             # 2 k-tiles over L
R = BS * T_ENC            # 25600 num-head rows per core
CHUNK = 512               # num-head rows per chunk
NCH = R // CHUNK          # 50 chunks
OPAD = 256                # y-projection moving width (>=256 keeps f32r fast)

# gate chunk order g, i, f, o: the c-update chain completes while o streams
GATE_ORDER = (2, 0, 1, 3)

PROFILE = False
LAST_RESULTS = None
LAST_IN_MAPS = None


def build_program(seq_len=100, use_gate_bias=False, use_out_bias=False):
    nc = bacc.Bacc("TRN2", target_bir_lowering=False, debug=False)

    d = {}
    def din(name, shape, dt=F32R):
        d[name] = nc.dram_tensor(name, shape, dt, kind="ExternalInput").ap()
    def dout(name, shape, dt=FP32):
        d[name] = nc.dram_tensor(name, shape, dt, kind="ExternalOutput").ap()

    din("wih_t", [KH, 128, 4 * H])        # (W_ih/2).T  k-tiled
    din("whh_t", [KH, 128, 4 * H])
    din("wout_t", [KH, 128, OPAD])        # (W_out/2).T zero-padded to OPAD
    din("wseq_t", [KL, 128, H])
    din("wl2h_t", [KL, 128, H])           # (2 W_lat2hid).T
    din("wl2h2_t", [KL, 128, H])
    din("w2_t", [KH, 128, 1])
    din("eh_t", [KL, 128, BS])            # encoder_hidden shard, transposed
    din("x_t", [KL, 128, R])              # encoder_outputs shard, transposed
    din("g0", [1, 4 * H])                 # relu(x0) @ W_ih.T + b_ih + b_hh
    din("bias_g", [1, 4 * H])             # b_ih + b_hh (emitted if nonzero)
    din("bias_out", [1, OPAD])
    din("bias_h0", [1, H])                # 2*b_lat2hid (K=1 row into h0 psum)
    din("bias_c0", [1, H])
    din("b_seq", [128, KH], FP32)         # b_seq laid out [128, 4] per h-tile
    din("b_seq2", [1, 1], FP32)

    dout("dec_out", [BS, seq_len, O])
    dout("h_out", [BS, H])
    dout("c_out", [BS, H])
    dout("num_out", [1, R])

    with tile.TileContext(nc) as tc:
        _emit(tc, nc, d, seq_len, use_gate_bias, use_out_bias)
    nc.compile()
    return nc


def _emit(tc, nc, d, seq_len, use_gate_bias, use_out_bias):
    import contextlib
    ctx = contextlib.ExitStack()
    with ctx:
        wpool = ctx.enter_context(tc.tile_pool(name="weights", bufs=1))
        spool = ctx.enter_context(tc.tile_pool(name="state", bufs=1))
        apool = ctx.enter_context(tc.tile_pool(name="acts", bufs=2))
        xpool = ctx.enter_context(tc.tile_pool(name="xenc", bufs=3))
        ypool = ctx.enter_context(tc.tile_pool(name="ystage", bufs=2))
        gps = ctx.enter_context(tc.tile_pool(name="gatesps", bufs=3, space="PSUM"))
        scr = ctx.enter_context(tc.tile_pool(name="scratch", bufs=5, space="PSUM"))

        AF = mybir.ActivationFunctionType
        ALU = mybir.AluOpType

        # ---- resident weights ----
        wih = wpool.tile([128, KH, 4 * H], F32R, tag="wih")
        whh = wpool.tile([128, KH, 4 * H], F32R, tag="whh")
        wout = wpool.tile([128, KH, OPAD], F32R, tag="wout")
        wseq = wpool.tile([128, KL, H], F32R, tag="wseq")
        wl2h = wpool.tile([128, KL, H], F32R, tag="wl2h")
        wl2h2 = wpool.tile([128, KL, H], F32R, tag="wl2h2")
        w2 = wpool.tile([128, KH, 1], F32R, tag="w2")
        eht = wpool.tile([128, KL, BS], F32R, tag="eht")
        g0 = wpool.tile([1, 4 * H], F32R, tag="g0")
        bseq = wpool.tile([128, KH], FP32, tag="bseq")
        bseq2 = wpool.tile([1, 1], FP32, tag="bseq2")
        for k in range(KL):
            nc.sync.dma_start(eht[:, k, :], d["eh_t"][k])
            nc.sync.dma_start(wl2h[:, k, :], d["wl2h_t"][k])
            nc.sync.dma_start(wl2h2[:, k, :], d["wl2h2_t"][k])
            nc.sync.dma_start(wseq[:, k, :], d["wseq_t"][k])
        for k in range(KH):
            nc.sync.dma_start(whh[:, k, :], d["whh_t"][k])
            nc.sync.dma_start(wih[:, k, :], d["wih_t"][k])
            nc.sync.dma_start(wout[:, k, :], d["wout_t"][k])
            nc.sync.dma_start(w2[:, k, :], d["w2_t"][k])
        nc.sync.dma_start(g0[:], d["g0"][:])
        nc.sync.dma_start(bseq[:], d["b_seq"][:])
        nc.sync.dma_start(bseq2[:], d["b_seq2"][:])

        bias_g = bias_out = None
        if use_gate_bias:
            bias_g = wpool.tile([1, 4 * H], F32R, tag="bias_g")
            nc.sync.dma_start(bias_g[:], d["bias_g"][:])
        if use_out_bias:
            bias_out = wpool.tile([1, OPAD], F32R, tag="bias_out")
            nc.sync.dma_start(bias_out[:], d["bias_out"][:])
        bias_h0 = wpool.tile([1, H], F32R, tag="bias_h0")
        bias_c0 = wpool.tile([1, H], F32R, tag="bias_c0")
        nc.sync.dma_start(bias_h0[:], d["bias_h0"][:])
        nc.sync.dma_start(bias_c0[:], d["bias_c0"][:])

        ident = wpool.tile([128, 128], FP32, tag="ident")
        make_identity(nc, ident[:])
        ones_f = wpool.tile([1, 128], FP32, tag="ones_f")
        nc.vector.memset(ones_f[:], 1.0)
        ones = wpool.tile([1, 128], F32R, tag="ones")
        nc.vector.tensor_copy(ones[:], ones_f[:])
        ident_r = wpool.tile([128, 128], F32R, tag="ident_r")
        nc.vector.tensor_copy(ident_r[:], ident[:])

        # ---- persistent state (H = 2h, D = 2c) ----
        hst = spool.tile([128, H], F32R, tag="hst")
        dst_c = spool.tile([128, H], FP32, tag="dst_c")
        hT = spool.tile([128, KH, 128], F32R, tag="hT")
        xT = spool.tile([128, KH, 128], F32R, tag="xT")

        def dve_lrelu(out_ap, ps_ap, bias=None):
            # out = lrelu(ps + bias); two DVE ops, no ACT LUT switch
            t = apool.tile([ps_ap.shape[0], ps_ap.shape[-1]], FP32, tag="lrtmp")
            if bias is None:
                nc.vector.tensor_copy(t[:], ps_ap)
            else:
                nc.vector.tensor_scalar(t[:], ps_ap, bias, None, ALU.add)
            nc.vector.scalar_tensor_tensor(out_ap, t[:], 0.01, t[:],
                                           ALU.mult, ALU.max)

        # ---- h0 / c0 (state enters pre-doubled via the 2x-scaled weights) ----
        for dst, w, brow in ((hst, wl2h, bias_h0), (dst_c, wl2h2, bias_c0)):
            ps = scr.tile([128, H], FP32, tag="scr")
            for k in range(KL):
                nc.tensor.matmul(ps[:], eht[:, k, :], w[:, k, :],
                                 start=(k == 0), stop=False)
            nc.tensor.matmul(ps[:], ones[:], brow[:], start=False, stop=True)
            dve_lrelu(dst[:], ps[:])

        for k in range(KH):
            tp = scr.tile([128, 128], F32R, tag="scr")
            nc.tensor.transpose(tp[:], hst[:, k * 128:(k + 1) * 128], ident_r[:])
            nc.scalar.activation(hT[:, k, :], tp[:], AF.Copy)

        # ---- num head machinery (software-pipelined into the scan) ----
        xe_tiles = {}
        act_tiles = {}
        nhalf_tiles = {}

        def prefetch_chunk(cidx):
            if cidx >= NCH:
                return
            xe = xpool.tile([128, KL, CHUNK], F32R, tag="xe")
            for k in range(KL):
                nc.sync.dma_start(
                    xe[:, k, :], d["x_t"][k][:, cidx * CHUNK:(cidx + 1) * CHUNK])
            xe_tiles[cidx] = xe

        def num_hs(cidx, m):
            # first-layer matmuls for one m-tile (PE); psum kept for the lrelu
            xe = xe_tiles[cidx]
            ps = scr.tile([128, CHUNK], FP32, tag="scr", name="numps")
            for k in range(KL):
                nc.tensor.matmul(ps[:], wseq[:, k, m * 128:(m + 1) * 128],
                                 xe[:, k, :], start=(k == 0), stop=(k == KL - 1))
            if m == KH - 1:
                del xe_tiles[cidx]
            return ps

        def num_lrelu(cidx, m, ps):
            a = apool.tile([128, CHUNK], F32R, tag="numact", bufs=4)
            dve_lrelu(a[:], ps[:], bias=bseq[:, m:m + 1])
            act_tiles[(cidx, m)] = a

        def num_tail_mm(pair):
            # second-layer accumulation (PE), one step behind the lrelus;
            # each half-pair finishes in its own psum slot (copied/reduced to
            # SBUF the same step, so nothing pins psum across steps)
            (cidx, m0), (_, m1) = pair
            nt = scr.tile([1, CHUNK], FP32, tag="scr", name="ntile")
            for j, m in enumerate((m0, m1)):
                a = act_tiles.pop((cidx, m))
                nc.tensor.matmul(nt[:], w2[:, m, :], a[:],
                                 start=(j == 0), stop=(j == 1))
            return cidx, m1, nt

        def num_tail_dve(cidx, m1, nt):
            if m1 == 1:
                nh = apool.tile([1, CHUNK], FP32, tag="nhalf", bufs=2)
                nc.vector.tensor_copy(nh[:], nt[:])
                nhalf_tiles[cidx] = nh
            else:
                nh = nhalf_tiles.pop(cidx)
                t2 = apool.tile([1, CHUNK], FP32, tag="ntmp", bufs=2)
                nc.vector.scalar_tensor_tensor(t2[:], nh[:], bseq2[0:1, 0:1],
                                               nt[:], ALU.add, ALU.add)
                no = ypool.tile([1, CHUNK], FP32, tag="numout")
                nc.vector.tensor_scalar(no[:], t2[:], 0.0, None, ALU.max)
                nc.sync.dma_start(
                    d["num_out"][:, cidx * CHUNK:(cidx + 1) * CHUNK], no[:])

        # chunks finished before the scan (they overlap the weight DMAs)
        PRE = min(0, NCH)
        prefetch_chunk(0)
        prefetch_chunk(1)
        for c in range(PRE):
            prefetch_chunk(c + 2)
            for half in range(2):
                pair = [(c, 2 * half), (c, 2 * half + 1)]
                for cc, m in pair:
                    num_lrelu(cc, m, num_hs(cc, m))
                num_tail_dve(*num_tail_mm(pair))
        pend = []

        # ---- the scan ----
        for t in range(seq_len):
            # gates = 2relu(h) @ (W_ih/2).T + 2h @ (W_hh/2).T (+ bias)
            # each chunk accumulates in its own rotating psum tile so the
            # tanh that consumes it has a precise, early dependency
            gch = {}
            for n in GATE_ORDER:
                gch[n] = gps.tile([128, 512], FP32, tag="gch", name="gch")
                gsl = gch[n][:]
                nsl = slice(n * 512, (n + 1) * 512)
                for k in range(KH):
                    nc.tensor.matmul(gsl, hT[:, k, :], whh[:, k, nsl],
                                     start=(k == 0), stop=False)
                if t == 0:
                    nc.tensor.matmul(gsl, ones[:], g0[:, nsl],
                                     start=False, stop=True)
                else:
                    if use_gate_bias:
                        nc.tensor.matmul(gsl, ones[:], bias_g[:, nsl],
                                         start=False, stop=False)
                    for k in range(KH):
                        nc.tensor.matmul(gsl, xT[:, k, :],
                                         wih[:, k, nsl], start=False,
                                         stop=(k == KH - 1))

            # first-layer num matmuls + deferred second layer fill the
            # gate->state dependency window on the PE
            cidx, half = divmod(t, 2)
            cidx += PRE
            work = [(cidx, 2 * half), (cidx, 2 * half + 1)] if cidx < NCH else []
            hs_ps = [num_hs(c, m) for c, m in work]
            if work and half == 0:
                prefetch_chunk(cidx + 2)
            tail = num_tail_mm(pend) if pend else None

            # tanh-only nonlinearities: sig(x) = (tanh(x/2)+1)/2
            th_g = apool.tile([128, 512], FP32, tag="th_g")
            th_i = apool.tile([128, 512], FP32, tag="th_i")
            th_f = apool.tile([128, 512], FP32, tag="th_f")
            nc.scalar.activation(th_g[:], gch[2][:], AF.Tanh)
            nc.scalar.activation(th_i[:], gch[0][:], AF.Tanh, scale=0.5)
            nc.scalar.activation(th_f[:], gch[1][:], AF.Tanh, scale=0.5)
            th_o = apool.tile([128, 512], FP32, tag="th_o")

            # D = 2c update: D' = 0.5(th_f+1)D + (th_i+1)th_g
            tmp = apool.tile([128, 512], FP32, tag="tmp")
            nc.vector.scalar_tensor_tensor(tmp[:], th_i[:], 1.0, th_g[:],
                                           ALU.add, ALU.mult)
            c2 = apool.tile([128, 512], FP32, tag="c2")
            nc.vector.scalar_tensor_tensor(c2[:], th_f[:], 1.0, dst_c[:],
                                           ALU.add, ALU.mult)
            nc.vector.scalar_tensor_tensor(dst_c[:], c2[:], 0.5, tmp[:],
                                           ALU.mult, ALU.add)
            th_c = apool.tile([128, 512], FP32, tag="th_c")
            nc.scalar.activation(th_c[:, 0:256], dst_c[:, 0:256], AF.Tanh, scale=0.5)
            nc.scalar.activation(th_c[:, 256:512], dst_c[:, 256:512], AF.Tanh, scale=0.5)
            nc.scalar.activation(th_o[:, 0:256], gch[3][:, 0:256], AF.Tanh, scale=0.5)
            nc.scalar.activation(th_o[:, 256:512], gch[3][:, 256:512], AF.Tanh, scale=0.5)
            # H = 2h = (th_o+1) tanh(c), in k-quarters so transposes start early
            for k in range(KH):
                ksl = slice(k * 128, (k + 1) * 128)
                nc.vector.scalar_tensor_tensor(hst[:, ksl], th_o[:, ksl], 1.0,
                                               th_c[:, ksl], ALU.add, ALU.mult)
            if tail is not None:
                num_tail_dve(*tail)

            # rebuild transposed state; copies alternate ACT/DVE so the y
            # matmuls are not serialized behind one engine
            tps = []
            for k in range(KH):
                tp = scr.tile([128, 128], F32R, tag="scr", name="tp")
                nc.tensor.transpose(tp[:], hst[:, k * 128:(k + 1) * 128], ident_r[:])
                if k % 2 == 0:
                    nc.scalar.activation(hT[:, k, :], tp[:], AF.Copy)
                else:
                    nc.vector.tensor_copy(hT[:, k, :], tp[:])
                tps.append(tp)

            # y = H @ (W_out/2).T  (moving side padded to OPAD for f32r rate)
            yps = scr.tile([128, OPAD], FP32, tag="scr", name="yps")
            for k in range(KH):
                nc.tensor.matmul(yps[:], hT[:, k, :], wout[:, k, :],
                                 start=(k == 0), stop=(k == KH - 1 and not use_out_bias))
            if use_out_bias:
                nc.tensor.matmul(yps[:], ones[:], bias_out[:], start=False, stop=True)

            if t + 1 < seq_len:
                for k in range(KH):
                    nc.vector.tensor_relu(xT[:, k, :], tps[k][:])
            y = ypool.tile([128, O], FP32, tag="y")
            nc.scalar.activation(y[:], yps[:, :O], AF.Copy)
            nc.sync.dma_start(d["dec_out"][:, t, :], y[:])
            for (c, m), ps in zip(work, hs_ps):
                num_lrelu(c, m, ps)
            pend = work

        # drain the deferred num tail (and any chunks past the scan)
        if pend:
            num_tail_dve(*num_tail_mm(pend))
        cidx = seq_len // 2 + PRE
        while cidx < NCH:
            prefetch_chunk(cidx)
            for half in range(2):
                pair = [(cidx, 2 * half), (cidx, 2 * half + 1)]
                for c, m in pair:
                    num_lrelu(c, m, num_hs(c, m))
                num_tail_dve(*num_tail_mm(pair))
            cidx += 1

        # outputs carry the true h = H/2, c = D/2
        ho = ypool.tile([128, H], FP32, tag="houtst")
        nc.vector.tensor_scalar(ho[:], hst[:].bitcast(FP32), 0.5, None, mybir.AluOpType.mult)
        nc.sync.dma_start(d["h_out"][:], ho[:])
        co = ypool.tile([128, H], FP32, tag="coutst")
        nc.vector.tensor_scalar(co[:], dst_c[:], 0.5, None, mybir.AluOpType.mult)
        nc.sync.dma_start(d["c_out"][:], co[:])


# ---------------------------------------------------------------------------
# host side
# ---------------------------------------------------------------------------

_CACHE = {}


def _prep_host(inputs):
    f32 = lambda a: np.ascontiguousarray(np.asarray(a), dtype=np.float32)
    W_ih, W_hh = f32(inputs["W_ih"]), f32(inputs["W_hh"])
    b_ih, b_hh = f32(inputs["b_ih"]), f32(inputs["b_hh"])
    W_out, b_out = f32(inputs["W_out"]), f32(inputs["b_out"])
    W_emb, b_emb = f32(inputs["W_emb"]), f32(inputs["b_emb"])
    W_seq, b_seq = f32(inputs["W_seq"]), f32(inputs["b_seq"])
    W_seq2, b_seq2 = f32(inputs["W_seq2"]), f32(inputs["b_seq2"])
    W_l2h, b_l2h = f32(inputs["W_lat2hid"]), f32(inputs["b_lat2hid"])
    W_l2h2, b_l2h2 = f32(inputs["W_lat2hid2"]), f32(inputs["b_lat2hid2"])
    start = f32(inputs["start_token"])

    x0 = start @ W_emb.T + b_emb                      # [1, H]
    bias_g = (b_ih + b_hh)[None, :]                   # [1, 4H]
    g0 = np.maximum(x0, 0.0) @ W_ih.T + bias_g        # [1, 4H] exact

    wout_pad = np.zeros((H, OPAD), np.float32)
    wout_pad[:, :O] = 0.5 * W_out.T
    bout_pad = np.zeros((1, OPAD), np.float32)
    bout_pad[0, :O] = b_out

    shared = {
        "wih_t": np.ascontiguousarray(0.5 * W_ih.T).reshape(KH, 128, 4 * H),
        "whh_t": np.ascontiguousarray(0.5 * W_hh.T).reshape(KH, 128, 4 * H),
        "wout_t": np.ascontiguousarray(wout_pad).reshape(KH, 128, OPAD),
        "wseq_t": np.ascontiguousarray(W_seq.T).reshape(KL, 128, H),
        "wl2h_t": np.ascontiguousarray(2.0 * W_l2h.T).reshape(KL, 128, H),
        "wl2h2_t": np.ascontiguousarray(2.0 * W_l2h2.T).reshape(KL, 128, H),
        "w2_t": np.ascontiguousarray(W_seq2.T).reshape(KH, 128, 1),
        "g0": np.ascontiguousarray(g0),
        "bias_g": np.ascontiguousarray(bias_g),
        "bias_out": bout_pad,
        "bias_h0": np.ascontiguousarray(2.0 * b_l2h[None, :]),
        "bias_c0": np.ascontiguousarray(2.0 * b_l2h2[None, :]),
        "b_seq": np.ascontiguousarray(b_seq.reshape(KH, 128).T),
        "b_seq2": np.ascontiguousarray(b_seq2.reshape(1, 1)),
    }
    flags = dict(
        use_gate_bias=bool(np.any(bias_g != 0)),
        use_out_bias=bool(np.any(b_out != 0)),
    )
    return shared, flags


def kernel(**inputs):
    enc = np.ascontiguousarray(np.asarray(inputs["encoder_outputs"]), dtype=np.float32)
    eh = np.ascontiguousarray(np.asarray(inputs["encoder_hidden"]), dtype=np.float32)
    seq_len = int(inputs["seq_len"])

    shared, flags = _prep_host(inputs)

    key = (seq_len, tuple(sorted(flags.items())))
    if key not in _CACHE:
        _CACHE[key] = build_program(seq_len=seq_len, **flags)
    nc = _CACHE[key]

    in_maps = []
    for cid in range(NCORES):
        sl = slice(cid * BS, (cid + 1) * BS)
        m = dict(shared)
        m["eh_t"] = np.ascontiguousarray(eh[sl].T).reshape(KL, 128, BS)
        m["x_t"] = np.ascontiguousarray(
            enc[sl].reshape(R, L).T).reshape(KL, 128, R)
        in_maps.append(m)

    global LAST_RESULTS, LAST_IN_MAPS
    LAST_IN_MAPS = in_maps
    res = run_bass_kernel_spmd(nc, in_maps, core_ids=list(range(NCORES)),
                               trace=PROFILE)
    LAST_RESULTS = res

    dec = np.concatenate([res.results[i]["dec_out"] for i in range(NCORES)], axis=0)
    hT = np.concatenate([res.results[i]["h_out"] for i in range(NCORES)], axis=0)
    cT = np.concatenate([res.results[i]["c_out"] for i in range(NCORES)], axis=0)
    num = np.concatenate(
        [res.results[i]["num_out"].reshape(BS, T_ENC) for i in range(NCORES)],
        axis=0)[..., None]
    return dec, hT[None], cT[None], num
